# revision 11
# baseline (speedup 1.0000x reference)
"""Contrastive (Cauchy-kernel InfoNCE) loss on 8 Trainium2 NeuronCores.

Math: for anchors a_i = features[i] (i < b) and the canonical full-batch
neighbor indices, the loss is

    loss = mean_i [ ln(S_i) + ln(1 + ||a_i - f_{i+b}||^2) ]
    S_i  = sum_{n != i} P[i, n],   P[i, n] = 1 / (1 + ||a_i - f_n||^2)

The device computes ONLY the probit row-sums S_i; everything cheap or
precision-critical lives on the host: ||f_n||^2 (exact f32), the positive
-pair distances, r_i = 1/(1+||a_i||^2), the final ln + mean.

Device program (per core, 128 anchors):
    bank[i, n] = a_i . f_n - ||f_n||^2 / 2          (fp8 DoubleRow matmuls)
    bank[i, i] += 2^16                              (tiny I-matmul: masks the
                                                     degenerate self column)
    Q[i, n] = 1/(bank * (-2 r_i) + 1) = (1+||a_i||^2) P[i, n]   (ACT recip,
                row-sums via the ACT accumulator -> out [128, 2])
Host: S_i = r_i * (sum of the two accumulator columns).

The feature matrix is shipped fp8 (e4m3) in the DoubleRow layout
[64, 2, 2048] (dims 0-63 plane 0, dims 64-127 plane 1), which runs the PE
at 2x bf16 rate and halves HBM traffic.  -||f||^2/2 is shipped as an fp8
hi+lo pair (residual splitting), giving bf16-grade accuracy through a
single DoubleRow matmul.  The gram stationary is the feature tile's own
first 128 columns (anchor block first via block permutation), so the only
per-core inputs are: features, the sq/ones row, the 256*I mask operand,
and the f32 ACT scale column (-2 r).

Sharding: data-parallel over anchors; core c owns anchors c*128..(c+1)*128.
Host sums ln() terms over all 8 cores' outputs.
"""

import numpy as np
import orjson

import concourse.bass as bass
import concourse.bass_isa as bass_isa
import concourse.bass2jax as bass2jax
import concourse.bass_utils as bass_utils
import concourse.mybir as mybir
import concourse.tile as tile
from concourse.masks import make_identity
from concourse.bass_utils import run_bass_kernel_spmd

B = 1024
DIM = 128
N = 2 * B            # 2048 feature rows
NCORES = 8
PB = B // NCORES     # 128 anchors per core
CH = 512             # psum bank / matmul chunk columns
F32 = mybir.dt.float32
BF16 = mybir.dt.bfloat16
FP8 = mybir.dt.float8e4   # e4m3
FP8NP = mybir.dt.np(FP8)
NJUNK_A = 6          # PE keep-alive matmuls before the output transpose
NJUNK_B = 10         # ... and after, bridging to the teardown
NJUNK_ACT = 2        # ACT keep-alive passes after the last accumulator read
MASKC = 128.0        # mask matmul operand; MASKC^2=16384 lands on the diag
                     # (this fp8 e4m3 variant is IEEE-style: max finite 240,
                     # 256 would round to inf and inf*0 NaN-poisons the PE)

REV = "v5"           # lands in a tile tag: busts the neuron-compile-cache
                     # for compiler-flag-only revisions

# Set by a driver to profile the HW execution (requires an NTFF hook).
TRACE = False
LAST_RESULT = None


def _split_multi_waits(bir_json: bytes) -> bytes:
    """The walrus build here accepts only ONE sync-wait per instruction,
    while Tile freely attaches several (one per producer proc). Engines pop
    their queues in order, so hoisting the extra waits onto injected NoOps
    immediately before the instruction is semantically identical."""
    m = orjson.loads(bir_json)
    changed = False
    for fn in m.get("functions", []):
        for blk in fn.get("blocks", []):
            out = []
            for inst in blk.get("instructions", []):
                si = inst.get("sync_info")
                ow = (si or {}).get("on_wait") or []
                if len(ow) > 1:
                    changed = True
                    for k, w in enumerate(ow[:-1]):
                        out.append(
                            {
                                "debug": inst.get("debug", 0),
                                "engine": inst["engine"],
                                "ins": [],
                                "outs": [],
                                "name": f"{inst['name']}-w{k}",
                                "opcode": "NoOp",
                                "text_hint": "wait_split",
                                "sync_info": {"on_update": [], "on_wait": [w]},
                            }
                        )
                    si["on_wait"] = [ow[-1]]
                if inst.get("op_name") == "EVENT_SEMAPHORE_RANGE_CLEAR":
                    inst["engine"] = "SP"
                    changed = True
                out.append(inst)
            blk["instructions"] = out
    return orjson.dumps(m) if changed else bir_json


def _patch_compiler():
    if getattr(bass_utils, "_wait_split_patch", False):
        return
    orig = bass_utils.compile_bir_kernel

    def patched(bir_json, tmpdir, neff_name="file.neff"):
        return orig(_split_multi_waits(bir_json), tmpdir, neff_name=neff_name)

    bass_utils.compile_bir_kernel = patched
    bass2jax.compile_bir_kernel = patched
    bass_utils._wait_split_patch = True


def _act_recip(nc, out, in_, scale, bias=1.0, accum_out=None):
    """ACT Reciprocal activation: out = 1/(in_*scale + bias).

    bass.activation() refuses Reciprocal outright (it has table-grade
    accuracy), but this loss only needs ~1e-3 on a 2047-term average, so
    emit the InstActivation directly. bias must be an immediate here
    (walrus sundagen requirement for Copy/Reciprocal); scale may be a
    per-partition [128,1] AP."""
    eng = nc.scalar
    inputs = [eng.lower_ap(in_)]
    for arg in (float(bias), scale, 0.0):
        if isinstance(arg, float):
            inputs.append(mybir.ImmediateValue(dtype=mybir.dt.float32, value=arg))
        else:
            inputs.append(eng.lower_ap(arg))
    outputs = [eng.lower_ap(out)]
    if accum_out is not None:
        outputs.append(eng.lower_ap(accum_out))
    return eng.add_instruction(
        mybir.InstActivation(
            name=nc.get_next_instruction_name(),
            func=mybir.ActivationFunctionType.Reciprocal,
            ins=inputs,
            outs=outputs,
        )
    )


def _build_v3():
    """Per-core program; see module docstring for the layout."""
    _patch_compiler()
    nc = bass.Bass(enable_partition_id=False)
    ftp = nc.dram_tensor("ftp", [64, 2, N], FP8, kind="ExternalInput")
    sqx = nc.dram_tensor("sqx", [1, 2, N + 128], FP8, kind="ExternalInput")
    imt = nc.dram_tensor("imt", [64, 2, 128], FP8, kind="ExternalInput")
    rct = nc.dram_tensor("rct", [128, 1], F32, kind="ExternalInput")
    outp = nc.dram_tensor("out", [2, 128], F32, kind="ExternalOutput")
    DR = mybir.MatmulPerfMode.DoubleRow

    with tile.TileContext(nc) as tc:
        with (
            tc.tile_pool(name="sb", bufs=1) as sb,
            tc.tile_pool(name="psum", bufs=1, space="PSUM") as psum,
        ):
            ft = sb.tile([64, 2, N], FP8, tag=f"ft_{REV}")
            sqo = sb.tile([1, 2, N + 128], FP8, tag="sqo")
            im = sb.tile([64, 2, 128], FP8, tag="im")
            rc = sb.tile([128, 1], F32, tag="rc")
            win = sb.tile([1, 1], F32, tag="win")
            recw = sb.tile([1, 1], F32, tag="recw")
            ident = sb.tile([128, 128], F32, tag="ident")
            qj0 = sb.tile([128, 2 * CH], BF16, tag="qj0")
            qj1a = sb.tile([128, 2 * CH], BF16, tag="qj1a")
            sparts = sb.tile([128, 2], F32, tag="sparts")
            spT = sb.tile([2, 128], F32, tag="spT")
            # separate PSUM tiles so the first probit pass depends only on
            # chunk 0's matmuls, not the whole bank
            bank0 = psum.tile([128, 2 * CH], F32, tag="bank0")
            bank1 = psum.tile([128, 2 * CH], F32, tag="bank1")
            junkb = psum.tile([128, CH], F32, tag="junkb")
            tpp = psum.tile([2, 128], F32, tag="tpp")

            # ACT queue: tiny rc DMA first, then the reciprocal-table warm
            # (~1.3us) so the table load overlaps the feature DMAs instead
            # of gating the first probit pass.
            nc.scalar.dma_start(out=rc[:, :], in_=rct[:, :])
            nc.vector.memset(win, 1.0)
            _act_recip(nc, recw, win, 1.0)

            # Two DMA rings, ordered by PE consumption: left feature half
            # first on SP, mask operand first on Pool. Each DMA's readiness
            # = last data + ~1us of per-queue completion-semaphore posts,
            # so the first matmul starts ~10us into the window.
            nc.sync.dma_start(out=ft[:, :, 0:1024], in_=ftp[:, :, 0:1024])
            nc.sync.dma_start(out=sqo[:, :, :], in_=sqx[:, :, :])
            nc.gpsimd.dma_start(out=im[:, :, :], in_=imt[:, :, :])
            nc.gpsimd.dma_start(out=ft[:, :, 1024:N], in_=ftp[:, :, 1024:N])
            make_identity(nc, ident)

            # PE, in input-readiness order and gap-free: the tensor engine
            # clock ramps (0.65 -> 1.2 -> 2.4 GHz) only under CONTINUOUS
            # execution, and the teardown's per-semaphore clear loop runs
            # at whatever clock the sequencer holds when it starts.
            ones = sqo[:, :, N:N + 128]
            anch = ft[:, :, 0:128]
            DRk = dict(perf_mode=DR)
            nc.tensor.matmul(bank0[:, 0:CH], anch, ft[:, :, 0:CH], start=True, stop=False, **DRk)
            nc.tensor.matmul(bank0[:, CH:2 * CH], anch, ft[:, :, CH:2 * CH], start=True, stop=False, **DRk)
            nc.tensor.matmul(bank0[:, 0:CH], ones, sqo[:, :, 0:CH], start=False, stop=False, **DRk)
            nc.tensor.matmul(bank0[:, CH:2 * CH], ones, sqo[:, :, CH:2 * CH], start=False, stop=True, **DRk)
            nc.tensor.matmul(bank0[:, 0:128], im, im, start=False, stop=True, **DRk)
            nc.tensor.matmul(bank1[:, 0:CH], anch, ft[:, :, 2 * CH:3 * CH], start=True, stop=False, **DRk)
            nc.tensor.matmul(bank1[:, CH:2 * CH], anch, ft[:, :, 3 * CH:N], start=True, stop=False, **DRk)
            nc.tensor.matmul(bank1[:, 0:CH], ones, sqo[:, :, 2 * CH:3 * CH], start=False, stop=True, **DRk)
            nc.tensor.matmul(bank1[:, CH:2 * CH], ones, sqo[:, :, 3 * CH:N], start=False, stop=True, **DRk)

            # ACT probits: Q = 1/(bank*(-2r) + 1); row-sums via the ACT
            # accumulator.
            _act_recip(nc, qj0, bank0[:, :], rc, 1.0, accum_out=sparts[:, 0:1])
            _act_recip(nc, qj1a, bank1[:, :], rc, 1.0, accum_out=sparts[:, 1:2])

            # PE keep-alive: junk matmuls bridge from the real work to the
            # teardown so the clock is ramped when the semaphore-clear loop
            # hits the Tensor sequencer (the teardown pacer: ~49 clears at
            # 115ns cold vs ~57ns ramped).
            for _ in range(NJUNK_A):
                nc.tensor.matmul(junkb[:, :], ones, sqo[:, :, 0:CH], start=True, stop=True, **DRk)

            # [128, 2] -> [2, 128] via PE transpose so the output DMA is 2
            # descriptors (2 completion posts) instead of 128 8-byte ones,
            # whose per-queue completion trickle costs ~2us.
            nc.tensor.transpose(tpp[:, :], sparts[:, :], ident)
            nc.vector.tensor_copy(spT[:, :], tpp[:, :])
            nc.sync.dma_start(out=outp[:, :], in_=spT[:, :])

            for _ in range(NJUNK_B):
                nc.tensor.matmul(junkb[:, :], ones, sqo[:, :, 0:CH], start=True, stop=True, **DRk)
            # ACT keep-alive, same idea for the Scalar sequencer's clears.
            for _ in range(NJUNK_ACT):
                _act_recip(nc, qj0[:, 0:CH], bank0[:, 0:CH], rc, 1.0)

    return nc


_NC = None


def _canonical_inds():
    idx = np.arange(B)
    not_self = ~np.eye(B, dtype=bool)
    neg1 = np.broadcast_to(idx[None, :], (B, B))[not_self].reshape(B, B - 1)
    neg2 = neg1 + B
    pos = (idx + B)[:, None]
    return np.concatenate([pos, neg1, neg2], axis=1)


_CANON = None


def _is_canonical(neigh_inds):
    global _CANON
    if neigh_inds.shape != (B, 2 * B - 1):
        return False
    if _CANON is None:
        _CANON = _canonical_inds()
    return np.array_equal(np.asarray(neigh_inds, dtype=np.int64), _CANON)


def _run_fast(feats):
    global _NC, LAST_RESULT

    if _NC is None:
        _NC = _build_v3()

    f64 = feats.astype(np.float64)
    sq64 = np.sum(f64 * f64, axis=1)                       # exact ||f_n||^2
    fq8 = feats.astype(FP8NP)

    # mask operand: 256 * I in the DoubleRow layout [64, 2, 128]
    imat = np.zeros((64, 2, 128), dtype=FP8NP)
    ii = np.arange(128)
    imat[ii % 64, ii // 64, ii] = np.asarray(MASKC, dtype=FP8NP)

    in_maps = []
    for c in range(NCORES):
        order = [c, NCORES + c] + [
            blk for blk in range(16) if blk not in (c, NCORES + c)
        ]
        rows = np.concatenate([np.arange(blk * 128, (blk + 1) * 128) for blk in order])
        # features, transposed + block-permuted, DoubleRow planes
        ftp = np.ascontiguousarray(
            fq8[rows].T.reshape(2, 64, N).transpose(1, 0, 2)
        )
        # -||f||^2/2 as fp8 hi+lo residual pair, ones block appended
        s = (-0.5 * sq64[rows]).astype(np.float32)
        s_hi = s.astype(FP8NP)
        s_lo = (s - s_hi.astype(np.float32)).astype(FP8NP)
        one128 = np.ones(128, dtype=FP8NP)
        sqxp = np.stack(
            [np.concatenate([s_hi, one128]), np.concatenate([s_lo, one128])]
        )[None]                                            # [1, 2, N+128]
        rct = (-2.0 / (1.0 + sq64[c * PB:(c + 1) * PB])).astype(np.float32)[:, None]
        in_maps.append(
            {"ftp": ftp, "sqx": np.ascontiguousarray(sqxp), "imt": imat, "rct": rct}
        )

    res = run_bass_kernel_spmd(_NC, in_maps, list(range(NCORES)), trace=TRACE)
    LAST_RESULT = res

    total = 0.0
    for c in range(NCORES):
        i = np.arange(c * PB, (c + 1) * PB)
        sp = np.asarray(res.results[c]["out"], dtype=np.float64)   # [2, 128]
        r = 1.0 / (1.0 + sq64[i])
        # subtract the analytic masked self-column residual (Q_ii ~ -0.005)
        aq = fq8[i].astype(np.float64)
        bank_ii = np.sum(aq * aq, axis=1) - 0.5 * sq64[i] + MASKC * MASKC
        qii = 1.0 / (bank_ii * (-2.0 * r) + 1.0)
        S = r * (sp.sum(axis=0) - qii)
        dpos1 = np.sum((f64[i] - f64[i + B]) ** 2, axis=1) + 1.0
        total += float(np.sum(np.log(S * dpos1)))
    return np.asarray(total / B, dtype=np.float32)


def _run_general(feats, neigh_inds):
    """Correctness fallback for non-canonical neighbor indices."""
    b = feats.shape[0] // 2
    origs = feats[:b]
    gram = origs @ feats.T
    sq = np.sum(feats * feats, axis=1)
    dists = sq[:b, None] + sq[None, :] - 2.0 * gram
    probs = 1.0 / (1.0 + dists)
    rows = np.arange(b)[:, None]
    sel = probs[rows, np.asarray(neigh_inds, dtype=np.int64)]
    loss = -(np.log(sel[:, 0]) - np.log(np.sum(sel, axis=1)))
    return np.asarray(np.mean(loss), dtype=np.float32)


def kernel(features, neigh_inds):
    feats = np.ascontiguousarray(np.asarray(features, dtype=np.float32))
    ni = np.asarray(neigh_inds)
    if _is_canonical(ni):
        return _run_fast(feats)
    return _run_general(feats, ni)


# revision 12
# speedup vs baseline: 1.0649x; 1.0649x over previous
"""Contrastive (Cauchy-kernel InfoNCE) loss on 8 Trainium2 NeuronCores.

Math: for anchors a_i = features[i] (i < b) and the canonical full-batch
neighbor indices, the loss is

    loss = mean_i [ ln(S_i) + ln(1 + ||a_i - f_{i+b}||^2) ]
    S_i  = sum_{n != i} P[i, n],   P[i, n] = 1 / (1 + ||a_i - f_n||^2)

The device computes ONLY the probit row-sums S_i; everything cheap or
precision-critical lives on the host: ||f_n||^2 (exact f32), the positive
-pair distances, r_i = 1/(1+||a_i||^2), the final ln + mean.

Device program (per core, 128 anchors):
    bank[i, n] = a_i . f_n - ||f_n||^2 / 2          (fp8 DoubleRow matmuls)
    bank[i, i] += 2^16                              (tiny I-matmul: masks the
                                                     degenerate self column)
    Q[i, n] = 1/(bank * (-2 r_i) + 1) = (1+||a_i||^2) P[i, n]   (ACT recip,
                row-sums via the ACT accumulator -> out [128, 2])
Host: S_i = r_i * (sum of the two accumulator columns).

The feature matrix is shipped fp8 (e4m3) in the DoubleRow layout
[64, 2, 2048] (dims 0-63 plane 0, dims 64-127 plane 1), which runs the PE
at 2x bf16 rate and halves HBM traffic.  -||f||^2/2 is shipped as an fp8
hi+lo pair (residual splitting), giving bf16-grade accuracy through a
single DoubleRow matmul.  The gram stationary is the feature tile's own
first 128 columns (anchor block first via block permutation), so the only
per-core inputs are: features, the sq/ones row, the 256*I mask operand,
and the f32 ACT scale column (-2 r).

Sharding: data-parallel over anchors; core c owns anchors c*128..(c+1)*128.
Host sums ln() terms over all 8 cores' outputs.
"""

import numpy as np
import orjson

import concourse.bass as bass
import concourse.bass_isa as bass_isa
import concourse.bass2jax as bass2jax
import concourse.bass_utils as bass_utils
import concourse.mybir as mybir
import concourse.tile as tile
from concourse.masks import make_identity
from concourse.bass_utils import run_bass_kernel_spmd

B = 1024
DIM = 128
N = 2 * B            # 2048 feature rows
NCORES = 8
PB = B // NCORES     # 128 anchors per core
CH = 512             # psum bank / matmul chunk columns
F32 = mybir.dt.float32
BF16 = mybir.dt.bfloat16
FP8 = mybir.dt.float8e4   # e4m3
FP8NP = mybir.dt.np(FP8)
MASKC = 128.0        # mask matmul operand; MASKC^2=16384 lands on the diag
                     # (this fp8 e4m3 variant is IEEE-style: max finite 240,
                     # 256 would round to inf and inf*0 NaN-poisons the PE)

REV = "v6"           # lands in a tile tag: busts the neuron-compile-cache
                     # for compiler-flag-only revisions

# Set by a driver to profile the HW execution (requires an NTFF hook).
TRACE = False
LAST_RESULT = None


def _split_multi_waits(bir_json: bytes) -> bytes:
    """The walrus build here accepts only ONE sync-wait per instruction,
    while Tile freely attaches several (one per producer proc). Engines pop
    their queues in order, so hoisting the extra waits onto injected NoOps
    immediately before the instruction is semantically identical."""
    m = orjson.loads(bir_json)
    changed = False
    for fn in m.get("functions", []):
        for blk in fn.get("blocks", []):
            out = []
            for inst in blk.get("instructions", []):
                si = inst.get("sync_info")
                ow = (si or {}).get("on_wait") or []
                if len(ow) > 1:
                    changed = True
                    for k, w in enumerate(ow[:-1]):
                        out.append(
                            {
                                "debug": inst.get("debug", 0),
                                "engine": inst["engine"],
                                "ins": [],
                                "outs": [],
                                "name": f"{inst['name']}-w{k}",
                                "opcode": "NoOp",
                                "text_hint": "wait_split",
                                "sync_info": {"on_update": [], "on_wait": [w]},
                            }
                        )
                    si["on_wait"] = [ow[-1]]
                if inst.get("op_name") == "EVENT_SEMAPHORE_RANGE_CLEAR":
                    inst["engine"] = "SP"
                    changed = True
                out.append(inst)
            blk["instructions"] = out
    return orjson.dumps(m) if changed else bir_json


def _patch_compiler():
    if getattr(bass_utils, "_wait_split_patch", False):
        return
    orig = bass_utils.compile_bir_kernel

    def patched(bir_json, tmpdir, neff_name="file.neff"):
        return orig(_split_multi_waits(bir_json), tmpdir, neff_name=neff_name)

    bass_utils.compile_bir_kernel = patched
    bass2jax.compile_bir_kernel = patched
    bass_utils._wait_split_patch = True


def _act_recip(nc, out, in_, scale, bias=1.0, accum_out=None):
    """ACT Reciprocal activation: out = 1/(in_*scale + bias).

    bass.activation() refuses Reciprocal outright (it has table-grade
    accuracy), but this loss only needs ~1e-3 on a 2047-term average, so
    emit the InstActivation directly. bias must be an immediate here
    (walrus sundagen requirement for Copy/Reciprocal); scale may be a
    per-partition [128,1] AP."""
    eng = nc.scalar
    inputs = [eng.lower_ap(in_)]
    for arg in (float(bias), scale, 0.0):
        if isinstance(arg, float):
            inputs.append(mybir.ImmediateValue(dtype=mybir.dt.float32, value=arg))
        else:
            inputs.append(eng.lower_ap(arg))
    outputs = [eng.lower_ap(out)]
    if accum_out is not None:
        outputs.append(eng.lower_ap(accum_out))
    return eng.add_instruction(
        mybir.InstActivation(
            name=nc.get_next_instruction_name(),
            func=mybir.ActivationFunctionType.Reciprocal,
            ins=inputs,
            outs=outputs,
        )
    )


def _build_v3():
    """Per-core program; see module docstring for the layout."""
    _patch_compiler()
    nc = bass.Bass(enable_partition_id=False)
    ftp = nc.dram_tensor("ftp", [64, 2, N], FP8, kind="ExternalInput")
    sqx = nc.dram_tensor("sqx", [1, 2, N + 128], FP8, kind="ExternalInput")
    imt = nc.dram_tensor("imt", [64, 2, 128], FP8, kind="ExternalInput")
    rct = nc.dram_tensor("rct", [128, 1], F32, kind="ExternalInput")
    outp = nc.dram_tensor("out", [2, 128], F32, kind="ExternalOutput")
    DR = mybir.MatmulPerfMode.DoubleRow

    with tile.TileContext(nc) as tc:
        with (
            tc.tile_pool(name="sb", bufs=1) as sb,
            tc.tile_pool(name="psum", bufs=1, space="PSUM") as psum,
        ):
            ft = sb.tile([64, 2, N], FP8, tag=f"ft_{REV}")
            sqo = sb.tile([1, 2, N + 128], FP8, tag="sqo")
            im = sb.tile([64, 2, 128], FP8, tag="im")
            rc = sb.tile([128, 1], F32, tag="rc")
            win = sb.tile([1, 1], F32, tag="win")
            recw = sb.tile([1, 1], F32, tag="recw")
            ident = sb.tile([128, 128], F32, tag="ident")
            qj0 = sb.tile([128, 2 * CH], BF16, tag="qj0")
            qj1a = sb.tile([128, 2 * CH], BF16, tag="qj1a")
            sparts = sb.tile([128, 2], F32, tag="sparts")
            spT = sb.tile([2, 128], F32, tag="spT")
            # separate PSUM tiles so the first probit pass depends only on
            # chunk 0's matmuls, not the whole bank
            bank0 = psum.tile([128, 2 * CH], F32, tag="bank0")
            bank1 = psum.tile([128, 2 * CH], F32, tag="bank1")
            tpp = psum.tile([2, 128], F32, tag="tpp")

            # ACT queue: tiny rc DMA first, then the reciprocal-table warm
            # (~1.3us) so the table load overlaps the feature DMAs instead
            # of gating the first probit pass.
            nc.scalar.dma_start(out=rc[:, :], in_=rct[:, :])
            nc.vector.memset(win, 1.0)
            _act_recip(nc, recw, win, 1.0)

            # Two DMA rings, ordered by PE consumption: left feature half
            # first on SP, mask operand first on Pool. Each DMA's readiness
            # = last data + ~1us of per-queue completion-semaphore posts,
            # so the first matmul starts ~10us into the window.
            nc.sync.dma_start(out=ft[:, :, 0:1024], in_=ftp[:, :, 0:1024])
            nc.sync.dma_start(out=sqo[:, :, :], in_=sqx[:, :, :])
            nc.gpsimd.dma_start(out=im[:, :, :], in_=imt[:, :, :])
            nc.gpsimd.dma_start(out=ft[:, :, 1024:N], in_=ftp[:, :, 1024:N])
            make_identity(nc, ident)

            # PE, in input-readiness order (ft_left, sqo, imat, ft_right)
            # so the ~427ns/matmul dispatch cadence runs gap-free.
            ones = sqo[:, :, N:N + 128]
            anch = ft[:, :, 0:128]
            DRk = dict(perf_mode=DR)
            nc.tensor.matmul(bank0[:, 0:CH], anch, ft[:, :, 0:CH], start=True, stop=False, **DRk)
            nc.tensor.matmul(bank0[:, CH:2 * CH], anch, ft[:, :, CH:2 * CH], start=True, stop=False, **DRk)
            nc.tensor.matmul(bank0[:, 0:CH], ones, sqo[:, :, 0:CH], start=False, stop=False, **DRk)
            nc.tensor.matmul(bank0[:, CH:2 * CH], ones, sqo[:, :, CH:2 * CH], start=False, stop=True, **DRk)
            nc.tensor.matmul(bank0[:, 0:128], im, im, start=False, stop=True, **DRk)
            nc.tensor.matmul(bank1[:, 0:CH], anch, ft[:, :, 2 * CH:3 * CH], start=True, stop=False, **DRk)
            nc.tensor.matmul(bank1[:, CH:2 * CH], anch, ft[:, :, 3 * CH:N], start=True, stop=False, **DRk)
            nc.tensor.matmul(bank1[:, 0:CH], ones, sqo[:, :, 2 * CH:3 * CH], start=False, stop=True, **DRk)
            nc.tensor.matmul(bank1[:, CH:2 * CH], ones, sqo[:, :, 3 * CH:N], start=False, stop=True, **DRk)

            # ACT probits: Q = 1/(bank*(-2r) + 1); row-sums via the ACT
            # accumulator.
            _act_recip(nc, qj0, bank0[:, :], rc, 1.0, accum_out=sparts[:, 0:1])
            _act_recip(nc, qj1a, bank1[:, :], rc, 1.0, accum_out=sparts[:, 1:2])

            # [128, 2] -> [2, 128] via PE transpose so the output DMA is 2
            # descriptors (2 completion posts) instead of 128 8-byte ones,
            # whose per-queue completion trickle costs ~2us.
            nc.tensor.transpose(tpp[:, :], sparts[:, :], ident)
            nc.vector.tensor_copy(spT[:, :], tpp[:, :])
            nc.sync.dma_start(out=outp[:, :], in_=spT[:, :])

    return nc


_NC = None


def _canonical_inds():
    idx = np.arange(B)
    not_self = ~np.eye(B, dtype=bool)
    neg1 = np.broadcast_to(idx[None, :], (B, B))[not_self].reshape(B, B - 1)
    neg2 = neg1 + B
    pos = (idx + B)[:, None]
    return np.concatenate([pos, neg1, neg2], axis=1)


_CANON = None


def _is_canonical(neigh_inds):
    global _CANON
    if neigh_inds.shape != (B, 2 * B - 1):
        return False
    if _CANON is None:
        _CANON = _canonical_inds()
    return np.array_equal(np.asarray(neigh_inds, dtype=np.int64), _CANON)


def _run_fast(feats):
    global _NC, LAST_RESULT

    if _NC is None:
        _NC = _build_v3()

    f64 = feats.astype(np.float64)
    sq64 = np.sum(f64 * f64, axis=1)                       # exact ||f_n||^2
    fq8 = feats.astype(FP8NP)

    # mask operand: 256 * I in the DoubleRow layout [64, 2, 128]
    imat = np.zeros((64, 2, 128), dtype=FP8NP)
    ii = np.arange(128)
    imat[ii % 64, ii // 64, ii] = np.asarray(MASKC, dtype=FP8NP)

    in_maps = []
    for c in range(NCORES):
        order = [c, NCORES + c] + [
            blk for blk in range(16) if blk not in (c, NCORES + c)
        ]
        rows = np.concatenate([np.arange(blk * 128, (blk + 1) * 128) for blk in order])
        # features, transposed + block-permuted, DoubleRow planes
        ftp = np.ascontiguousarray(
            fq8[rows].T.reshape(2, 64, N).transpose(1, 0, 2)
        )
        # -||f||^2/2 as fp8 hi+lo residual pair, ones block appended
        s = (-0.5 * sq64[rows]).astype(np.float32)
        s_hi = s.astype(FP8NP)
        s_lo = (s - s_hi.astype(np.float32)).astype(FP8NP)
        one128 = np.ones(128, dtype=FP8NP)
        sqxp = np.stack(
            [np.concatenate([s_hi, one128]), np.concatenate([s_lo, one128])]
        )[None]                                            # [1, 2, N+128]
        rct = (-2.0 / (1.0 + sq64[c * PB:(c + 1) * PB])).astype(np.float32)[:, None]
        in_maps.append(
            {"ftp": ftp, "sqx": np.ascontiguousarray(sqxp), "imt": imat, "rct": rct}
        )

    res = run_bass_kernel_spmd(_NC, in_maps, list(range(NCORES)), trace=TRACE)
    LAST_RESULT = res

    total = 0.0
    for c in range(NCORES):
        i = np.arange(c * PB, (c + 1) * PB)
        sp = np.asarray(res.results[c]["out"], dtype=np.float64)   # [2, 128]
        r = 1.0 / (1.0 + sq64[i])
        # subtract the analytic masked self-column residual (Q_ii ~ -0.005)
        aq = fq8[i].astype(np.float64)
        bank_ii = np.sum(aq * aq, axis=1) - 0.5 * sq64[i] + MASKC * MASKC
        qii = 1.0 / (bank_ii * (-2.0 * r) + 1.0)
        S = r * (sp.sum(axis=0) - qii)
        dpos1 = np.sum((f64[i] - f64[i + B]) ** 2, axis=1) + 1.0
        total += float(np.sum(np.log(S * dpos1)))
    return np.asarray(total / B, dtype=np.float32)


def _run_general(feats, neigh_inds):
    """Correctness fallback for non-canonical neighbor indices."""
    b = feats.shape[0] // 2
    origs = feats[:b]
    gram = origs @ feats.T
    sq = np.sum(feats * feats, axis=1)
    dists = sq[:b, None] + sq[None, :] - 2.0 * gram
    probs = 1.0 / (1.0 + dists)
    rows = np.arange(b)[:, None]
    sel = probs[rows, np.asarray(neigh_inds, dtype=np.int64)]
    loss = -(np.log(sel[:, 0]) - np.log(np.sum(sel, axis=1)))
    return np.asarray(np.mean(loss), dtype=np.float32)


def kernel(features, neigh_inds):
    feats = np.ascontiguousarray(np.asarray(features, dtype=np.float32))
    ni = np.asarray(neigh_inds)
    if _is_canonical(ni):
        return _run_fast(feats)
    return _run_general(feats, ni)


# revision 13
# speedup vs baseline: 1.1983x; 1.1252x over previous
"""Contrastive (Cauchy-kernel InfoNCE) loss on 8 Trainium2 NeuronCores.

Math: for anchors a_i = features[i] (i < b) and the canonical full-batch
neighbor indices, the loss is

    loss = mean_i [ ln(S_i) + ln(1 + ||a_i - f_{i+b}||^2) ]
    S_i  = sum_{n != i} P[i, n],   P[i, n] = 1 / (1 + ||a_i - f_n||^2)

The device computes ONLY the probit row-sums S_i; everything cheap or
precision-critical lives on the host: ||f_n||^2 (exact f32), the positive
-pair distances, r_i = 1/(1+||a_i||^2), the final ln + mean.

Device program (per core, 128 anchors):
    bank[i, n] = a_i . f_n - ||f_n||^2 / 2          (fp8 DoubleRow matmuls)
    bank[i, i] += 2^16                              (tiny I-matmul: masks the
                                                     degenerate self column)
    Q[i, n] = 1/(bank * (-2 r_i) + 1) = (1+||a_i||^2) P[i, n]   (ACT recip,
                row-sums via the ACT accumulator -> out [128, 2])
Host: S_i = r_i * (sum of the two accumulator columns).

The feature matrix is shipped fp8 (e4m3) in the DoubleRow layout
[64, 2, 2048] (dims 0-63 plane 0, dims 64-127 plane 1), which runs the PE
at 2x bf16 rate and halves HBM traffic.  -||f||^2/2 is shipped as an fp8
hi+lo pair (residual splitting), giving bf16-grade accuracy through a
single DoubleRow matmul.  The gram stationary is the feature tile's own
first 128 columns (anchor block first via block permutation), so the only
per-core inputs are: features, the sq/ones row, the 256*I mask operand,
and the f32 ACT scale column (-2 r).

Sharding: data-parallel over anchors; core c owns anchors c*128..(c+1)*128.
Host sums ln() terms over all 8 cores' outputs.
"""

import numpy as np
import orjson

import concourse.bass as bass
import concourse.bass_isa as bass_isa
import concourse.bass2jax as bass2jax
import concourse.bass_utils as bass_utils
import concourse.mybir as mybir
import concourse.tile as tile
from concourse.masks import make_identity
from concourse.bass_utils import run_bass_kernel_spmd

B = 1024
DIM = 128
N = 2 * B            # 2048 feature rows
NCORES = 8
PB = B // NCORES     # 128 anchors per core
CH = 512             # psum bank / matmul chunk columns
F32 = mybir.dt.float32
BF16 = mybir.dt.bfloat16
FP8 = mybir.dt.float8e4   # e4m3
FP8NP = mybir.dt.np(FP8)
MASKC = 128.0        # mask matmul operand; MASKC^2=16384 lands on the diag
                     # (this fp8 e4m3 variant is IEEE-style: max finite 240,
                     # 256 would round to inf and inf*0 NaN-poisons the PE)

REV = "v7"           # lands in a tile tag: busts the neuron-compile-cache
                     # for compiler-flag-only revisions

# Set by a driver to profile the HW execution (requires an NTFF hook).
TRACE = False
LAST_RESULT = None


def _split_multi_waits(bir_json: bytes) -> bytes:
    """The walrus build here accepts only ONE sync-wait per instruction,
    while Tile freely attaches several (one per producer proc). Engines pop
    their queues in order, so hoisting the extra waits onto injected NoOps
    immediately before the instruction is semantically identical."""
    m = orjson.loads(bir_json)
    changed = False
    for fn in m.get("functions", []):
        for blk in fn.get("blocks", []):
            out = []
            for inst in blk.get("instructions", []):
                si = inst.get("sync_info")
                ow = (si or {}).get("on_wait") or []
                if len(ow) > 1:
                    changed = True
                    for k, w in enumerate(ow[:-1]):
                        out.append(
                            {
                                "debug": inst.get("debug", 0),
                                "engine": inst["engine"],
                                "ins": [],
                                "outs": [],
                                "name": f"{inst['name']}-w{k}",
                                "opcode": "NoOp",
                                "text_hint": "wait_split",
                                "sync_info": {"on_update": [], "on_wait": [w]},
                            }
                        )
                    si["on_wait"] = [ow[-1]]
                if inst.get("op_name") == "EVENT_SEMAPHORE_RANGE_CLEAR":
                    inst["engine"] = "SP"
                    changed = True
                out.append(inst)
            blk["instructions"] = out
    return orjson.dumps(m) if changed else bir_json


def _patch_compiler():
    if getattr(bass_utils, "_wait_split_patch", False):
        return
    orig = bass_utils.compile_bir_kernel

    def patched(bir_json, tmpdir, neff_name="file.neff"):
        return orig(_split_multi_waits(bir_json), tmpdir, neff_name=neff_name)

    bass_utils.compile_bir_kernel = patched
    bass2jax.compile_bir_kernel = patched
    bass_utils._wait_split_patch = True


def _act_recip(nc, out, in_, scale, bias=1.0, accum_out=None):
    """ACT Reciprocal activation: out = 1/(in_*scale + bias).

    bass.activation() refuses Reciprocal outright (it has table-grade
    accuracy), but this loss only needs ~1e-3 on a 2047-term average, so
    emit the InstActivation directly. bias must be an immediate here
    (walrus sundagen requirement for Copy/Reciprocal); scale may be a
    per-partition [128,1] AP."""
    eng = nc.scalar
    inputs = [eng.lower_ap(in_)]
    for arg in (float(bias), scale, 0.0):
        if isinstance(arg, float):
            inputs.append(mybir.ImmediateValue(dtype=mybir.dt.float32, value=arg))
        else:
            inputs.append(eng.lower_ap(arg))
    outputs = [eng.lower_ap(out)]
    if accum_out is not None:
        outputs.append(eng.lower_ap(accum_out))
    return eng.add_instruction(
        mybir.InstActivation(
            name=nc.get_next_instruction_name(),
            func=mybir.ActivationFunctionType.Reciprocal,
            ins=inputs,
            outs=outputs,
        )
    )


def _build_v3():
    """Per-core program; see module docstring for the layout."""
    _patch_compiler()
    nc = bass.Bass(enable_partition_id=False)
    ftp = nc.dram_tensor("ftp", [64, 2, N], FP8, kind="ExternalInput")
    sqx = nc.dram_tensor("sqx", [1, 2, N + 128], FP8, kind="ExternalInput")
    imt = nc.dram_tensor("imt", [64, 2, 128], FP8, kind="ExternalInput")
    rct = nc.dram_tensor("rct", [128, 1], F32, kind="ExternalInput")
    outp = nc.dram_tensor("out", [2, 128], F32, kind="ExternalOutput")
    DR = mybir.MatmulPerfMode.DoubleRow

    with tile.TileContext(nc) as tc:
        with (
            tc.tile_pool(name="sb", bufs=1) as sb,
            tc.tile_pool(name="psum", bufs=1, space="PSUM") as psum,
        ):
            ft = sb.tile([64, 2, N], FP8, tag=f"ft_{REV}")
            sqo = sb.tile([1, 2, N + 128], FP8, tag="sqo")
            im = sb.tile([64, 2, 128], FP8, tag="im")
            rc = sb.tile([128, 1], F32, tag="rc")
            win = sb.tile([1, 1], F32, tag="win")
            recw = sb.tile([1, 1], F32, tag="recw")
            ident = sb.tile([128, 128], F32, tag="ident")
            qj0 = sb.tile([128, CH], BF16, tag="qj0")
            qj1a = sb.tile([128, 3 * CH], BF16, tag="qj1a")
            sparts = sb.tile([128, 2], F32, tag="sparts")
            spT = sb.tile([2, 128], F32, tag="spT")
            # separate PSUM tiles so the first probit pass depends only on
            # chunk 0's matmuls, not the whole bank
            bank0 = psum.tile([128, CH], F32, tag="bank0")
            bank1 = psum.tile([128, 3 * CH], F32, tag="bank1")
            tpp = psum.tile([2, 128], F32, tag="tpp")

            # Three DMA rings in parallel (issue ~0.7us each, ring latency
            # ~0.75us, then a ~1.3us per-queue completion-semaphore trickle
            # for any >=16-descriptor transfer): SP carries the left feature
            # half, Pool the sq row + mask operand, ACT the right half + the
            # ACT scale column. The reciprocal-table warm (~1.3us) follows
            # on the ACT queue, still well before the first probit pass.
            nc.sync.dma_start(out=ft[:, :, 0:1024], in_=ftp[:, :, 0:1024])
            nc.gpsimd.dma_start(out=sqo[:, :, :], in_=sqx[:, :, :])
            nc.gpsimd.dma_start(out=im[:, :, :], in_=imt[:, :, :])
            nc.scalar.dma_start(out=ft[:, :, 1024:N], in_=ftp[:, :, 1024:N])
            nc.scalar.dma_start(out=rc[:, :], in_=rct[:, :])
            nc.vector.memset(win, 1.0)
            _act_recip(nc, recw, win, 1.0)
            make_identity(nc, ident)

            # PE, in input-readiness order (sqo/imat first, then the feature
            # halves), ~427ns dispatch cadence per matmul. bank0 holds chunk
            # 0 only, so the first probit pass launches after 3 matmuls.
            ones = sqo[:, :, N:N + 128]
            anch = ft[:, :, 0:128]
            DRk = dict(perf_mode=DR)
            nc.tensor.matmul(bank0[:, :], ones, sqo[:, :, 0:CH], start=True, stop=False, **DRk)
            nc.tensor.matmul(bank0[:, 0:128], im, im, start=False, stop=False, **DRk)
            for j in range(1, 4):
                nc.tensor.matmul(
                    bank1[:, (j - 1) * CH:j * CH], ones, sqo[:, :, j * CH:(j + 1) * CH],
                    start=True, stop=False, **DRk,
                )
            nc.tensor.matmul(bank0[:, :], anch, ft[:, :, 0:CH], start=False, stop=True, **DRk)
            for j in range(1, 4):
                nc.tensor.matmul(
                    bank1[:, (j - 1) * CH:j * CH], anch, ft[:, :, j * CH:(j + 1) * CH],
                    start=False, stop=(j > 0), **DRk,
                )

            # ACT probits: Q = 1/(bank*(-2r) + 1); row-sums via the ACT
            # accumulator (read-acc between the passes).
            _act_recip(nc, qj0, bank0[:, :], rc, 1.0, accum_out=sparts[:, 0:1])
            _act_recip(nc, qj1a, bank1[:, :], rc, 1.0, accum_out=sparts[:, 1:2])

            # [128, 2] -> [2, 128] via PE transpose so the output DMA is 2
            # descriptors (2 completion posts) instead of 128 8-byte ones,
            # whose per-queue completion trickle costs ~2us.
            nc.tensor.transpose(tpp[:, :], sparts[:, :], ident)
            nc.vector.tensor_copy(spT[:, :], tpp[:, :])
            nc.sync.dma_start(out=outp[:, :], in_=spT[:, :])

    return nc


_NC = None


def _canonical_inds():
    idx = np.arange(B)
    not_self = ~np.eye(B, dtype=bool)
    neg1 = np.broadcast_to(idx[None, :], (B, B))[not_self].reshape(B, B - 1)
    neg2 = neg1 + B
    pos = (idx + B)[:, None]
    return np.concatenate([pos, neg1, neg2], axis=1)


_CANON = None


def _is_canonical(neigh_inds):
    global _CANON
    if neigh_inds.shape != (B, 2 * B - 1):
        return False
    if _CANON is None:
        _CANON = _canonical_inds()
    return np.array_equal(np.asarray(neigh_inds, dtype=np.int64), _CANON)


def _run_fast(feats):
    global _NC, LAST_RESULT

    if _NC is None:
        _NC = _build_v3()

    f64 = feats.astype(np.float64)
    sq64 = np.sum(f64 * f64, axis=1)                       # exact ||f_n||^2
    fq8 = feats.astype(FP8NP)

    # mask operand: 256 * I in the DoubleRow layout [64, 2, 128]
    imat = np.zeros((64, 2, 128), dtype=FP8NP)
    ii = np.arange(128)
    imat[ii % 64, ii // 64, ii] = np.asarray(MASKC, dtype=FP8NP)

    in_maps = []
    for c in range(NCORES):
        order = [c, NCORES + c] + [
            blk for blk in range(16) if blk not in (c, NCORES + c)
        ]
        rows = np.concatenate([np.arange(blk * 128, (blk + 1) * 128) for blk in order])
        # features, transposed + block-permuted, DoubleRow planes
        ftp = np.ascontiguousarray(
            fq8[rows].T.reshape(2, 64, N).transpose(1, 0, 2)
        )
        # -||f||^2/2 as fp8 hi+lo residual pair, ones block appended
        s = (-0.5 * sq64[rows]).astype(np.float32)
        s_hi = s.astype(FP8NP)
        s_lo = (s - s_hi.astype(np.float32)).astype(FP8NP)
        one128 = np.ones(128, dtype=FP8NP)
        sqxp = np.stack(
            [np.concatenate([s_hi, one128]), np.concatenate([s_lo, one128])]
        )[None]                                            # [1, 2, N+128]
        rct = (-2.0 / (1.0 + sq64[c * PB:(c + 1) * PB])).astype(np.float32)[:, None]
        in_maps.append(
            {"ftp": ftp, "sqx": np.ascontiguousarray(sqxp), "imt": imat, "rct": rct}
        )

    res = run_bass_kernel_spmd(_NC, in_maps, list(range(NCORES)), trace=TRACE)
    LAST_RESULT = res

    total = 0.0
    for c in range(NCORES):
        i = np.arange(c * PB, (c + 1) * PB)
        sp = np.asarray(res.results[c]["out"], dtype=np.float64)   # [2, 128]
        r = 1.0 / (1.0 + sq64[i])
        # subtract the analytic masked self-column residual (Q_ii ~ -0.005)
        aq = fq8[i].astype(np.float64)
        bank_ii = np.sum(aq * aq, axis=1) - 0.5 * sq64[i] + MASKC * MASKC
        qii = 1.0 / (bank_ii * (-2.0 * r) + 1.0)
        S = r * (sp.sum(axis=0) - qii)
        dpos1 = np.sum((f64[i] - f64[i + B]) ** 2, axis=1) + 1.0
        total += float(np.sum(np.log(S * dpos1)))
    return np.asarray(total / B, dtype=np.float32)


def _run_general(feats, neigh_inds):
    """Correctness fallback for non-canonical neighbor indices."""
    b = feats.shape[0] // 2
    origs = feats[:b]
    gram = origs @ feats.T
    sq = np.sum(feats * feats, axis=1)
    dists = sq[:b, None] + sq[None, :] - 2.0 * gram
    probs = 1.0 / (1.0 + dists)
    rows = np.arange(b)[:, None]
    sel = probs[rows, np.asarray(neigh_inds, dtype=np.int64)]
    loss = -(np.log(sel[:, 0]) - np.log(np.sum(sel, axis=1)))
    return np.asarray(np.mean(loss), dtype=np.float32)


def kernel(features, neigh_inds):
    feats = np.ascontiguousarray(np.asarray(features, dtype=np.float32))
    ni = np.asarray(neigh_inds)
    if _is_canonical(ni):
        return _run_fast(feats)
    return _run_general(feats, ni)


# revision 14
# speedup vs baseline: 1.2393x; 1.0342x over previous
"""Contrastive (Cauchy-kernel InfoNCE) loss on 8 Trainium2 NeuronCores.

Math: for anchors a_i = features[i] (i < b) and the canonical full-batch
neighbor indices, the loss is

    loss = mean_i [ ln(S_i) + ln(1 + ||a_i - f_{i+b}||^2) ]
    S_i  = sum_{n != i} P[i, n],   P[i, n] = 1 / (1 + ||a_i - f_n||^2)

The device computes ONLY the probit row-sums S_i; everything cheap or
precision-critical lives on the host: ||f_n||^2 (exact f32), the positive
-pair distances, r_i = 1/(1+||a_i||^2), the final ln + mean.

Device program (per core, 128 anchors):
    bank[i, n] = a_i . f_n - ||f_n||^2 / 2          (fp8 DoubleRow matmuls)
    bank[i, i] += 2^16                              (tiny I-matmul: masks the
                                                     degenerate self column)
    Q[i, n] = 1/(bank * (-2 r_i) + 1) = (1+||a_i||^2) P[i, n]   (ACT recip,
                row-sums via the ACT accumulator -> out [128, 2])
Host: S_i = r_i * (sum of the two accumulator columns).

The feature matrix is shipped fp8 (e4m3) in the DoubleRow layout
[64, 2, 2048] (dims 0-63 plane 0, dims 64-127 plane 1), which runs the PE
at 2x bf16 rate and halves HBM traffic.  -||f||^2/2 is shipped as an fp8
hi+lo pair (residual splitting), giving bf16-grade accuracy through a
single DoubleRow matmul.  The gram stationary is the feature tile's own
first 128 columns (anchor block first via block permutation), so the only
per-core inputs are: features, the sq/ones row, the 256*I mask operand,
and the f32 ACT scale column (-2 r).

Sharding: data-parallel over anchors; core c owns anchors c*128..(c+1)*128.
Host sums ln() terms over all 8 cores' outputs.
"""

import numpy as np
import orjson

import concourse.bass as bass
import concourse.bass_isa as bass_isa
import concourse.bass2jax as bass2jax
import concourse.bass_utils as bass_utils
import concourse.mybir as mybir
import concourse.tile as tile
from concourse.masks import make_identity
from concourse.bass_utils import run_bass_kernel_spmd

B = 1024
DIM = 128
N = 2 * B            # 2048 feature rows
NCORES = 8
PB = B // NCORES     # 128 anchors per core
CH = 512             # psum bank / matmul chunk columns
F32 = mybir.dt.float32
BF16 = mybir.dt.bfloat16
FP8 = mybir.dt.float8e4   # e4m3
FP8NP = mybir.dt.np(FP8)
MASKC = 128.0        # mask matmul operand; MASKC^2=16384 lands on the diag
                     # (this fp8 e4m3 variant is IEEE-style: max finite 240,
                     # 256 would round to inf and inf*0 NaN-poisons the PE)

REV = "v8"           # lands in a tile tag: busts the neuron-compile-cache
                     # for compiler-flag-only revisions

# Set by a driver to profile the HW execution (requires an NTFF hook).
TRACE = False
LAST_RESULT = None


def _split_multi_waits(bir_json: bytes) -> bytes:
    """The walrus build here accepts only ONE sync-wait per instruction,
    while Tile freely attaches several (one per producer proc). Engines pop
    their queues in order, so hoisting the extra waits onto injected NoOps
    immediately before the instruction is semantically identical."""
    m = orjson.loads(bir_json)
    changed = False
    for fn in m.get("functions", []):
        for blk in fn.get("blocks", []):
            out = []
            for inst in blk.get("instructions", []):
                si = inst.get("sync_info")
                ow = (si or {}).get("on_wait") or []
                if len(ow) > 1:
                    changed = True
                    for k, w in enumerate(ow[:-1]):
                        out.append(
                            {
                                "debug": inst.get("debug", 0),
                                "engine": inst["engine"],
                                "ins": [],
                                "outs": [],
                                "name": f"{inst['name']}-w{k}",
                                "opcode": "NoOp",
                                "text_hint": "wait_split",
                                "sync_info": {"on_update": [], "on_wait": [w]},
                            }
                        )
                    si["on_wait"] = [ow[-1]]
                if inst.get("op_name") == "EVENT_SEMAPHORE_RANGE_CLEAR":
                    inst["engine"] = "SP"
                    changed = True
                out.append(inst)
            blk["instructions"] = out
    return orjson.dumps(m) if changed else bir_json


def _patch_compiler():
    if getattr(bass_utils, "_wait_split_patch", False):
        return
    orig = bass_utils.compile_bir_kernel

    def patched(bir_json, tmpdir, neff_name="file.neff"):
        return orig(_split_multi_waits(bir_json), tmpdir, neff_name=neff_name)

    bass_utils.compile_bir_kernel = patched
    bass2jax.compile_bir_kernel = patched
    bass_utils._wait_split_patch = True


def _act_recip(nc, out, in_, scale, bias=1.0, accum_out=None):
    """ACT Reciprocal activation: out = 1/(in_*scale + bias).

    bass.activation() refuses Reciprocal outright (it has table-grade
    accuracy), but this loss only needs ~1e-3 on a 2047-term average, so
    emit the InstActivation directly. bias must be an immediate here
    (walrus sundagen requirement for Copy/Reciprocal); scale may be a
    per-partition [128,1] AP."""
    eng = nc.scalar
    inputs = [eng.lower_ap(in_)]
    for arg in (float(bias), scale, 0.0):
        if isinstance(arg, float):
            inputs.append(mybir.ImmediateValue(dtype=mybir.dt.float32, value=arg))
        else:
            inputs.append(eng.lower_ap(arg))
    outputs = [eng.lower_ap(out)]
    if accum_out is not None:
        outputs.append(eng.lower_ap(accum_out))
    return eng.add_instruction(
        mybir.InstActivation(
            name=nc.get_next_instruction_name(),
            func=mybir.ActivationFunctionType.Reciprocal,
            ins=inputs,
            outs=outputs,
        )
    )


def _build_v3():
    """Per-core program; see module docstring for the layout."""
    _patch_compiler()
    nc = bass.Bass(enable_partition_id=False)
    ftp = nc.dram_tensor("ftp", [64, 2, N], FP8, kind="ExternalInput")
    sqx = nc.dram_tensor("sqx", [1, 2, N + 128], FP8, kind="ExternalInput")
    imt = nc.dram_tensor("imt", [64, 2, 128], FP8, kind="ExternalInput")
    rct = nc.dram_tensor("rct", [128, 1], F32, kind="ExternalInput")
    outp = nc.dram_tensor("out", [2, 128], F32, kind="ExternalOutput")
    DR = mybir.MatmulPerfMode.DoubleRow

    with tile.TileContext(nc) as tc:
        with (
            tc.tile_pool(name="sb", bufs=1) as sb,
            tc.tile_pool(name="psum", bufs=1, space="PSUM") as psum,
        ):
            ft = sb.tile([64, 2, N], FP8, tag=f"ft_{REV}")
            sqo = sb.tile([1, 2, N + 128], FP8, tag="sqo")
            im = sb.tile([64, 2, 128], FP8, tag="im")
            rc = sb.tile([128, 1], F32, tag="rc")
            win = sb.tile([1, 1], F32, tag="win")
            recw = sb.tile([1, 1], F32, tag="recw")
            ident = sb.tile([128, 128], F32, tag="ident")
            qj0 = sb.tile([128, CH], BF16, tag="qj0")
            qj1a = sb.tile([128, 3 * CH], BF16, tag="qj1a")
            sparts = sb.tile([128, 2], F32, tag="sparts")
            spT = sb.tile([2, 128], F32, tag="spT")
            # separate PSUM tiles so the first probit pass depends only on
            # chunk 0's matmuls, not the whole bank
            bank0 = psum.tile([128, CH], F32, tag="bank0")
            bank1 = psum.tile([128, 3 * CH], F32, tag="bank1")
            tpp = psum.tile([2, 128], F32, tag="tpp")

            # Three DMA rings in parallel, ordered by what gates the PE:
            # sqo first on SP (it opens the matmul pipeline), the feature
            # halves split between SP and the ACT ring, the mask operand +
            # scale column on Pool. Issue ~0.7us each, ring latency ~0.75us,
            # then a ~1.3us per-queue completion-semaphore trickle for any
            # >=16-descriptor transfer.
            nc.sync.dma_start(out=sqo[:, :, :], in_=sqx[:, :, :])
            nc.sync.dma_start(out=ft[:, :, 1024:N], in_=ftp[:, :, 1024:N])
            nc.scalar.dma_start(out=ft[:, :, 0:1024], in_=ftp[:, :, 0:1024])
            nc.gpsimd.dma_start(out=im[:, :, :], in_=imt[:, :, :])
            nc.gpsimd.dma_start(out=rc[:, :], in_=rct[:, :])
            nc.vector.memset(win, 1.0)
            _act_recip(nc, recw, win, 1.0)
            make_identity(nc, ident)

            # PE, in input-readiness order at the ~427ns dispatch cadence:
            # three sq chunks, the mask (imat lands ~1.2us after sqo), the
            # last sq chunk, then the grams as the feature halves land.
            # bank0 holds chunk 0 only -> probit pass 0 launches right
            # after gram 0.
            ones = sqo[:, :, N:N + 128]
            anch = ft[:, :, 0:128]
            DRk = dict(perf_mode=DR)
            nc.tensor.matmul(bank0[:, :], ones, sqo[:, :, 0:CH], start=True, stop=False, **DRk)
            nc.tensor.matmul(bank1[:, 0:CH], ones, sqo[:, :, CH:2 * CH], start=True, stop=False, **DRk)
            nc.tensor.matmul(bank1[:, CH:2 * CH], ones, sqo[:, :, 2 * CH:3 * CH], start=True, stop=False, **DRk)
            nc.tensor.matmul(bank0[:, 0:128], im, im, start=False, stop=False, **DRk)
            nc.tensor.matmul(bank1[:, 2 * CH:3 * CH], ones, sqo[:, :, 3 * CH:N], start=True, stop=False, **DRk)
            nc.tensor.matmul(bank0[:, :], anch, ft[:, :, 0:CH], start=False, stop=True, **DRk)
            nc.tensor.matmul(bank1[:, 0:CH], anch, ft[:, :, CH:2 * CH], start=False, stop=True, **DRk)
            nc.tensor.matmul(bank1[:, CH:2 * CH], anch, ft[:, :, 2 * CH:3 * CH], start=False, stop=True, **DRk)
            nc.tensor.matmul(bank1[:, 2 * CH:3 * CH], anch, ft[:, :, 3 * CH:N], start=False, stop=True, **DRk)

            # ACT probits: Q = 1/(bank*(-2r) + 1); row-sums via the ACT
            # accumulator (read-acc between the passes).
            _act_recip(nc, qj0, bank0[:, :], rc, 1.0, accum_out=sparts[:, 0:1])
            _act_recip(nc, qj1a, bank1[:, :], rc, 1.0, accum_out=sparts[:, 1:2])

            # [128, 2] -> [2, 128] via PE transpose so the output DMA is 2
            # descriptors (2 completion posts) instead of 128 8-byte ones,
            # whose per-queue completion trickle costs ~2us.
            nc.tensor.transpose(tpp[:, :], sparts[:, :], ident)
            nc.vector.tensor_copy(spT[:, :], tpp[:, :])
            nc.sync.dma_start(out=outp[:, :], in_=spT[:, :])

    return nc


_NC = None


def _canonical_inds():
    idx = np.arange(B)
    not_self = ~np.eye(B, dtype=bool)
    neg1 = np.broadcast_to(idx[None, :], (B, B))[not_self].reshape(B, B - 1)
    neg2 = neg1 + B
    pos = (idx + B)[:, None]
    return np.concatenate([pos, neg1, neg2], axis=1)


_CANON = None


def _is_canonical(neigh_inds):
    global _CANON
    if neigh_inds.shape != (B, 2 * B - 1):
        return False
    if _CANON is None:
        _CANON = _canonical_inds()
    return np.array_equal(np.asarray(neigh_inds, dtype=np.int64), _CANON)


def _run_fast(feats):
    global _NC, LAST_RESULT

    if _NC is None:
        _NC = _build_v3()

    f64 = feats.astype(np.float64)
    sq64 = np.sum(f64 * f64, axis=1)                       # exact ||f_n||^2
    fq8 = feats.astype(FP8NP)

    # mask operand: 256 * I in the DoubleRow layout [64, 2, 128]
    imat = np.zeros((64, 2, 128), dtype=FP8NP)
    ii = np.arange(128)
    imat[ii % 64, ii // 64, ii] = np.asarray(MASKC, dtype=FP8NP)

    in_maps = []
    for c in range(NCORES):
        order = [c, NCORES + c] + [
            blk for blk in range(16) if blk not in (c, NCORES + c)
        ]
        rows = np.concatenate([np.arange(blk * 128, (blk + 1) * 128) for blk in order])
        # features, transposed + block-permuted, DoubleRow planes
        ftp = np.ascontiguousarray(
            fq8[rows].T.reshape(2, 64, N).transpose(1, 0, 2)
        )
        # -||f||^2/2 as fp8 hi+lo residual pair, ones block appended
        s = (-0.5 * sq64[rows]).astype(np.float32)
        s_hi = s.astype(FP8NP)
        s_lo = (s - s_hi.astype(np.float32)).astype(FP8NP)
        one128 = np.ones(128, dtype=FP8NP)
        sqxp = np.stack(
            [np.concatenate([s_hi, one128]), np.concatenate([s_lo, one128])]
        )[None]                                            # [1, 2, N+128]
        rct = (-2.0 / (1.0 + sq64[c * PB:(c + 1) * PB])).astype(np.float32)[:, None]
        in_maps.append(
            {"ftp": ftp, "sqx": np.ascontiguousarray(sqxp), "imt": imat, "rct": rct}
        )

    res = run_bass_kernel_spmd(_NC, in_maps, list(range(NCORES)), trace=TRACE)
    LAST_RESULT = res

    total = 0.0
    for c in range(NCORES):
        i = np.arange(c * PB, (c + 1) * PB)
        sp = np.asarray(res.results[c]["out"], dtype=np.float64)   # [2, 128]
        r = 1.0 / (1.0 + sq64[i])
        # subtract the analytic masked self-column residual (Q_ii ~ -0.005)
        aq = fq8[i].astype(np.float64)
        bank_ii = np.sum(aq * aq, axis=1) - 0.5 * sq64[i] + MASKC * MASKC
        qii = 1.0 / (bank_ii * (-2.0 * r) + 1.0)
        S = r * (sp.sum(axis=0) - qii)
        dpos1 = np.sum((f64[i] - f64[i + B]) ** 2, axis=1) + 1.0
        total += float(np.sum(np.log(S * dpos1)))
    return np.asarray(total / B, dtype=np.float32)


def _run_general(feats, neigh_inds):
    """Correctness fallback for non-canonical neighbor indices."""
    b = feats.shape[0] // 2
    origs = feats[:b]
    gram = origs @ feats.T
    sq = np.sum(feats * feats, axis=1)
    dists = sq[:b, None] + sq[None, :] - 2.0 * gram
    probs = 1.0 / (1.0 + dists)
    rows = np.arange(b)[:, None]
    sel = probs[rows, np.asarray(neigh_inds, dtype=np.int64)]
    loss = -(np.log(sel[:, 0]) - np.log(np.sum(sel, axis=1)))
    return np.asarray(np.mean(loss), dtype=np.float32)


def kernel(features, neigh_inds):
    feats = np.ascontiguousarray(np.asarray(features, dtype=np.float32))
    ni = np.asarray(neigh_inds)
    if _is_canonical(ni):
        return _run_fast(feats)
    return _run_general(feats, ni)
